# revision 1
# baseline (speedup 1.0000x reference)
"""Multi-head self-attention on 8 Trainium2 NeuronCores — v3 (pipelined).

Sharding: core c handles batch b = c // 4 and head-quad g = c % 4
(heads 4g..4g+3 = 256 of the 1024 projection columns). Each core runs
its heads' Q/K/V projections, attention, and a partial output projection
(contraction over its 256 context dims); the host sums 4 partials per
batch and adds bo' = bo + bv @ Wo (bv commutes with the normalized
softmax weights, so it folds into the output bias).

Pipeline design (per core), driven by the TRN2 engine cost model:
  - PE total ~164us of matmul rows is the floor; ScalarE holds ~133us of
    exp. The schedule keeps PE continuously busy and nests the exp
    stream inside it.
  - Phase 1 computes only pair0's Q/K (m=0) so attention(pair0) starts
    at ~16us. V(p0), Q/K(p1), V(p1) and the output projection are
    emitted as PE filler inside the attention loop's slack.
  - PSUM budget (8 banks): scores 2x[128,1024] double-buffered (4) +
    ctx 2x[65,512] single-buffered (2) + filler [128,512] x2 (2).
  - Attention iterates (pair, tq, sb): one exp instr [128,1024] covers
    both heads of the pair for a 512-wide t-window.
  - x/Wq/Wk/Wv stream in bf16 (half DMA bytes, same PE rate into f32
    psum); weights load as single consolidated DMAs to cut HWDGE issue
    serialization.
"""

import sys

sys.path.insert(0, "/opt/trn_rl_repo")

import numpy as np

B, T, D = 2, 2048, 1024
H = 16
DK = 64
NCORES = 8
HPC = 4            # heads per core
HD = HPC * DK      # 256 projection cols per core
P = 128
NT = 512           # matmul moving free dim
KB = D // P        # 8 contraction blocks for projections
MB = HD // P       # 2 col-blocks (head pairs) per core
SB = T // P        # 16 s-blocks
TQ = T // NT       # 4 t-quarters
VW = 65            # V columns per head incl. the denominator ones column
TH = T // 2

_CACHE = {}


def _build():
    import concourse.tile as tile
    from concourse import bacc, mybir

    f32 = mybir.dt.float32
    f32r = mybir.dt.float32r
    bf16 = mybir.dt.bfloat16
    Exp = mybir.ActivationFunctionType.Exp

    nc = bacc.Bacc("TRN2", target_bir_lowering=False, debug=False)

    xT_d = nc.dram_tensor("xT", [D, T], bf16, kind="ExternalInput").ap()
    wq_d = nc.dram_tensor("wq", [D, HD], bf16, kind="ExternalInput").ap()
    wk_d = nc.dram_tensor("wk", [D, HD], bf16, kind="ExternalInput").ap()
    wv_d = nc.dram_tensor("wv", [D, HD], bf16, kind="ExternalInput").ap()
    wo_d = nc.dram_tensor("wo", [HD, D], f32r, kind="ExternalInput").ap()
    bq_d = nc.dram_tensor("bq", [P, MB], f32, kind="ExternalInput").ap()
    bk_d = nc.dram_tensor("bk", [P, MB], f32, kind="ExternalInput").ap()
    out_d = nc.dram_tensor("out", [T, D], bf16, kind="ExternalOutput").ap()

    with tile.TileContext(nc) as tc:
        with tc.tile_pool(name="persist", bufs=1) as persist:
            QT = persist.tile([P, MB * T], f32r, tag="qt")
            KT = persist.tile([P, MB * T], f32r, tag="kt")
            Vn = persist.tile([P, HPC * SB * VW], f32r, tag="vn")
            ctxT_h = [
                persist.tile([P, MB * TH], f32r, tag=f"ctxt{i}", name=f"ctxT_{i}")
                for i in range(2)
            ]
            wo_sb = persist.tile([P, MB * D], f32r, tag="wo")
            xT_sb = persist.tile([P, KB * T], bf16, tag="xt")
            w_sb = {
                name: persist.tile([P, KB * HD], bf16, tag=f"w{name}", name=f"w_{name}")
                for name in ("q", "k", "v")
            }
            b_sb = {
                name: persist.tile([P, MB], f32, tag=f"b{name}", name=f"b_{name}")
                for name in ("q", "k")
            }
            nc.gpsimd.memset(Vn[:].bitcast(f32), 1.0)
            # warm the Exp activation table at t~0 (ScalarE idles until the
            # first scores land; the auto-inserted table load would otherwise
            # cost 1.3us right before the first real exp)
            warm = persist.tile([1, 2], f32, tag="warm")
            nc.vector.memset(warm[:], 0.0)
            nc.scalar.activation(warm[:], warm[:], Exp)

            # ---- DMA issue order = consumption order ----
            # wq/wk as single consolidated DMAs (one HWDGE slot each), then
            # the 8 xT k-blocks that pace the Q/K sweeps, then biases
            # (needed at first eviction), wv, wo.
            nc.sync.dma_start(w_sb["q"][:, 0:HD], wq_d[0:P, :])
            nc.sync.dma_start(xT_sb[:, 0:T // 2], xT_d[0:P, 0:T // 2])
            nc.sync.dma_start(w_sb["k"][:, 0:HD], wk_d[0:P, :])
            nc.sync.dma_start(xT_sb[:, T // 2:T], xT_d[0:P, T // 2:T])
            nc.sync.dma_start(
                w_sb["q"][:, HD:].rearrange("p (k j) -> p k j", k=KB - 1),
                wq_d[P:, :].rearrange("(k p) j -> p k j", p=P),
            )
            nc.sync.dma_start(xT_sb[:, T:2 * T], xT_d[P:2 * P, :])
            nc.sync.dma_start(
                w_sb["k"][:, HD:].rearrange("p (k j) -> p k j", k=KB - 1),
                wk_d[P:, :].rearrange("(k p) j -> p k j", p=P),
            )
            for k in range(2, KB):
                nc.sync.dma_start(
                    xT_sb[:, k * T:(k + 1) * T], xT_d[k * P:(k + 1) * P, :]
                )
            nc.sync.dma_start(b_sb["q"][:], bq_d[:])
            nc.sync.dma_start(b_sb["k"][:], bk_d[:])
            nc.sync.dma_start(
                w_sb["v"][:].rearrange("p (k j) -> p k j", k=KB),
                wv_d.rearrange("(k p) j -> p k j", p=P),
            )
            for m in range(MB):
                nc.sync.dma_start(wo_sb[:, m * D:(m + 1) * D], wo_d[m * P:(m + 1) * P, :])

            dests = {"q": QT, "k": KT}

            def qk_evict(name, m, n, pst):
                dst = dests[name][:, m * T + n * NT: m * T + (n + 1) * NT]
                nc.vector.tensor_scalar_add(dst, pst[:], b_sb[name][:, m:m + 1])

            Vn_r = Vn[:].rearrange("p (h sw) -> p h sw", h=HPC)

            def v_units(pair, pool, tag, q4s, evict_eng="scalar"):
                # V in natural [s, d] layout: per psum tile 4 mt-blocks x 128
                # of the pair's d-cols; one unit per (tile, k) + one eviction
                # (a single 4D copy into Vn's 65-stride head blocks).
                for q4 in q4s:
                    ps = pool.tile([P, NT], f32, tag=tag, name=f"v_{pair}_{q4}")

                    # one accumulation group at a time per psum bank: each
                    # mt-block's k-sweep is a sequential group (427ns unit)
                    def mk(j, q4=q4, ps=ps):
                        def emit():
                            mt = q4 * 4 + j
                            for k in range(KB):
                                nc.tensor.matmul(
                                    ps[:, j * P:(j + 1) * P],
                                    xT_sb[:, k * T + mt * P: k * T + (mt + 1) * P],
                                    w_sb["v"][:, k * HD + pair * P: k * HD + (pair + 1) * P],
                                    start=(k == 0),
                                    stop=(k == KB - 1),
                                )
                        return 427, emit

                    for j in range(4):
                        yield mk(j)

                    def evict(q4=q4, ps=ps):
                        h0 = 2 * pair
                        dst = (
                            Vn_r[:, h0:h0 + 2, q4 * 4 * VW:(q4 + 1) * 4 * VW]
                            .rearrange("p h (j w) -> p h j w", j=4)[:, :, :, 0:DK]
                        )
                        # prefetch tiles evict on the idle ScalarE; in-loop
                        # tiles evict on DVE (ScalarE is exp-saturated there)
                        src_ap = ps[:].rearrange("p (j h w) -> p h j w", j=4, h=2)
                        if evict_eng == "scalar":
                            nc.scalar.copy(dst, src_ap)
                        else:
                            nc.vector.tensor_copy(dst, src_ap)
                    yield 0, evict

            # ---------------- Phase 1: pair0 Q/K sweeps ----------------
            with tc.tile_pool(name="qk0_ps", bufs=8, space="PSUM") as qk0_ps:
                # alloc order = slot order: V0/V1 (allocs 9/10) land on the
                # slots of (k,0)/(q,0), whose evicts run first on DVE.
                final_order = [("k", 0), ("q", 0), ("k", 1), ("k", 2),
                               ("k", 3), ("q", 1), ("q", 2), ("q", 3)]
                pst0 = {
                    (name, n): qk0_ps.tile([P, NT], f32, tag="qk0", name=f"p0_{name}_{n}")
                    for name, n in final_order
                }
                for k in range(KB):
                    pairs = (
                        final_order if k == KB - 1
                        else [(nm, n) for nm in ("q", "k") for n in range(TQ)]
                    )
                    for name, n in pairs:
                        nc.tensor.matmul(
                            pst0[(name, n)][:],
                            w_sb[name][:, k * HD: k * HD + P],
                            xT_sb[:, k * T + n * NT: k * T + (n + 1) * NT],
                            start=(k == 0),
                            stop=(k == KB - 1),
                        )
                # evict order: (k,n0)/(q,n0) unblock scores(sb0); K blocks
                # n1-3 are needed within tq0's first 16 iters; QT n1-3 later.
                qk_evict("k", 0, 0, pst0[("k", 0)])
                qk_evict("q", 0, 0, pst0[("q", 0)])
                for n in range(1, TQ):
                    qk_evict("k", 0, n, pst0[("k", n)])
                for n in range(1, TQ):
                    qk_evict("q", 0, n, pst0[("q", n)])
                # V(p0) tiles 0-1 reuse freed qk0 slots; tiles 2-3 go through
                # the fill chain inside the attention loop.
                for _, u in v_units(0, qk0_ps, "qk0", range(2)):
                    u()

            # ---------------- Phase 2: attention + filler ----------------
            with (
                tc.tile_pool(name="exp", bufs=8) as exp_pool,
                tc.tile_pool(name="norm", bufs=8) as norm_pool,
                tc.tile_pool(name="outsb", bufs=8) as out_sb_pool,
                tc.tile_pool(name="score_ps", bufs=2, space="PSUM") as score_ps,
                tc.tile_pool(name="ctx_ps", bufs=2, space="PSUM") as ctx_ps,
                tc.tile_pool(name="fill_ps", bufs=2, space="PSUM") as fill_ps,
            ):
                # --- filler unit generators (each yields closures emitting
                # a small batch of PE work) ---
                def qk1_units(pairs):
                    # pair1 Q/K: k-pair units (2 matmuls, ~427ns) per (name, n)
                    for name, n in pairs:
                        if True:
                            ps = fill_ps.tile([P, NT], f32, tag="fill", name=f"qk1_{name}_{n}")

                            def mk(kp, name=name, n=n, ps=ps):
                                def emit():
                                    for k in (2 * kp, 2 * kp + 1):
                                        nc.tensor.matmul(
                                            ps[:],
                                            w_sb[name][:, k * HD + P: k * HD + 2 * P],
                                            xT_sb[:, k * T + n * NT: k * T + (n + 1) * NT],
                                            start=(k == 0),
                                            stop=(k == KB - 1),
                                        )
                                    if kp == KB // 2 - 1:
                                        qk_evict(name, 1, n, ps)
                                return 427, emit

                            for kp in range(KB // 2):
                                yield mk(kp)

                def out_units(tq, alt_pool=False):
                    # output projection for t-quarter tq: one unit per
                    # (mt block, 512-col half) for smooth 427ns pacing.
                    # alt_pool: alternate psum between the fill and ctx
                    # slot families (ctx banks are free after the last
                    # norm), doubling tail slot parallelism.
                    for j in range(4):
                        mt = tq * 4 + j
                        th, tt = mt // 8, mt % 8
                        ot_box = []

                        ps_box = {}

                        def emit(ne, m, mt=mt, th=th, tt=tt, tq=tq,
                                 ot_box=ot_box, ps_box=ps_box):
                            if ne == 0 and m == 0:
                                ot_box.append(
                                    out_sb_pool.tile([P, D], bf16, tag="osb", name=f"osb_{mt}")
                                )
                            ot = ot_box[0]
                            if m == 0:
                                pool, ptag = (
                                    (ctx_ps, "ctx") if (alt_pool and mt % 4 >= 2)
                                    else (fill_ps, "fill")
                                )
                                ps_box[ne] = pool.tile(
                                    [P, NT], f32, tag=ptag, name=f"o_{mt}_{ne}"
                                )
                            ps = ps_box[ne]
                            nc.tensor.matmul(
                                ps[:],
                                ctxT_h[th][:, m * TH + tt * P: m * TH + (tt + 1) * P],
                                wo_sb[:, m * D + ne * NT: m * D + (ne + 1) * NT],
                                start=(m == 0),
                                stop=(m == MB - 1),
                            )
                            if m == MB - 1:
                                if ne == 1 and tq >= TQ - 1:
                                    # after-loop tail only: ScalarE is idle
                                    # there. tq2's units run DURING tq3's
                                    # iters where ScalarE still streams exps
                                    # - a copy there delays the tail-gating
                                    # final exp. (GPSIMD cannot read PSUM.)
                                    nc.scalar.copy(ot[:, NT:], ps[:])
                                else:
                                    nc.vector.tensor_copy(ot[:, ne * NT:(ne + 1) * NT], ps[:])
                                if ne == 1:
                                    nc.sync.dma_start(out_d[mt * P:(mt + 1) * P, :], ot[:])
                        for ne in range(2):
                            for m in range(MB):
                                yield 213, (lambda ne=ne, m=m: emit(ne, m))

                # Fill chain for pair0's slack: V(p0) tiles 2-3 first (their
                # evictions must be emitted before ctx(sb8)/ctx(sb12)), then
                # QK(p1) and V(p1) interleaved; the leftover spills into
                # pair1-tq0 (which has no out-proj filler yet).
                import itertools

                def roundrobin2(a, b):
                    a, b = iter(a), iter(b)
                    while True:
                        ua = next(a, None)
                        ub = next(b, None)
                        if ua is None and ub is None:
                            return
                        if ua is not None:
                            yield ua
                        if ub is not None:
                            yield ub

                # deadline order: V(p1) tile q is due at pair1 iter 4q+1,
                # KT(p1) block n at iter 64+4n+... ; QT(p1) n1-3 are due a
                # whole tq later, so they may spill into pair1-tq0's slack
                fill_p0 = itertools.chain(
                    v_units(0, fill_ps, "fill", range(2, 4), evict_eng="vector"),
                    qk1_units([("k", 0), ("q", 0)]),
                    v_units(1, fill_ps, "fill", [0]),
                    qk1_units([("k", 1)]),
                    v_units(1, fill_ps, "fill", [1]),
                    qk1_units([("k", 2)]),
                    v_units(1, fill_ps, "fill", [2]),
                    qk1_units([("k", 3)]),
                    v_units(1, fill_ps, "fill", [3]),
                    qk1_units([("q", 1), ("q", 2), ("q", 3)]),
                )
                p0_spent, P0_RATE = 0.0, 280.0

                for pair in range(MB):
                    hs = (2 * pair, 2 * pair + 1)
                    for tq in range(TQ):
                        th, t0 = tq // 2, (tq % 2) * NT
                        cps = {
                            h: ctx_ps.tile([VW, NT], f32, tag="ctx", name=f"ctx_{h}")
                            for h in hs
                        }
                        fill_p1 = iter(out_units(tq - 1)) if (pair == 1 and tq > 0) else iter(())
                        p1_spent = 0.0

                        prev_exp = None
                        for sb in range(SB):
                            sc = score_ps.tile([P, 2 * NT], f32, tag="sc", name="sc")
                            for i, h in enumerate(hs):
                                po = (h % 2) * DK
                                nc.tensor.matmul(
                                    sc[:, i * NT:(i + 1) * NT],
                                    KT[po:po + DK, pair * T + sb * P: pair * T + (sb + 1) * P],
                                    QT[po:po + DK, pair * T + tq * NT:(pair * T) + (tq + 1) * NT],
                                    start=True,
                                    stop=True,
                                )
                            e = exp_pool.tile([P, 2 * NT], f32r, tag="exp", name="exp")
                            nc.scalar.activation(e[:], sc[:], Exp, scale=0.125)
                            # filler before ctx(prev): PE chews it while the
                            # previous tq's norm chain releases the ctx banks
                            if pair == 0:
                                it = tq * SB + sb
                                while p0_spent < (it + 1) * P0_RATE:
                                    cu = next(fill_p0, None)
                                    if cu is None:
                                        break
                                    p0_spent += max(cu[0], 107)
                                    cu[1]()
                            else:
                                while p1_spent < (sb + 1) * 185:
                                    cu = next(fill_p0, None) or next(fill_p1, None)
                                    if cu is None:
                                        break
                                    p1_spent += max(cu[0], 107)
                                    cu[1]()
                            if prev_exp is not None:
                                psb, pe = prev_exp
                                for i, h in enumerate(hs):
                                    nc.tensor.matmul(
                                        cps[h][:],
                                        Vn[:, (h * SB + psb) * VW: (h * SB + psb + 1) * VW],
                                        pe[:, i * NT:(i + 1) * NT],
                                        start=(psb == 0),
                                        stop=False,
                                    )
                            prev_exp = (sb, e)
                        # one filler unit BEFORE the epilogue ctx: the final
                        # ctx waits for the last exp; this keeps the in-order
                        # PE queue busy through part of that wait
                        psb, pe = prev_exp
                        for i, h in enumerate(hs):
                            nc.tensor.matmul(
                                cps[h][:],
                                Vn[:, (h * SB + psb) * VW: (h * SB + psb + 1) * VW],
                                pe[:, i * NT:(i + 1) * NT],
                                start=False,
                                stop=True,
                            )
                        if pair == 1:
                            for _, u in fill_p1:
                                u()

                        # normalize this t-quarter into ctxT
                        if pair == 1 and tq == TQ - 1:
                            # tail: chunk the multiplies per mt-block so the
                            # final out-proj units start as soon as their
                            # 128-col slice of ctxT is written
                            recs, bcs = {}, {}
                            for h in hs:
                                recs[h] = norm_pool.tile([1, NT], f32, tag="rec", name=f"rec_{h}")
                                bcs[h] = norm_pool.tile([DK, NT], f32, tag="bc", name=f"bc_{h}")
                            for c in range(NT // P):
                                for h in hs:
                                    po = (h % 2) * DK
                                    c0 = pair * TH + t0 + c * P
                                    cs = slice(c * P, (c + 1) * P)
                                    nc.vector.reciprocal(recs[h][:, cs], cps[h][DK:DK + 1, cs])
                                    nc.gpsimd.partition_broadcast(bcs[h][:, cs], recs[h][:, cs])
                                    nc.vector.tensor_mul(
                                        ctxT_h[th][po:po + DK, c0:c0 + P],
                                        cps[h][0:DK, cs],
                                        bcs[h][:, cs],
                                    )
                        else:
                            # recs back-to-back on DVE, bcs on Pool, then
                            # muls: avoids DVE head-of-line blocking behind
                            # each Pool broadcast
                            recs, bcs = {}, {}
                            for h in hs:
                                recs[h] = norm_pool.tile([1, NT], f32, tag="rec", name=f"rec_{h}")
                                nc.vector.reciprocal(recs[h][:], cps[h][DK:DK + 1, :])
                            for h in hs:
                                bcs[h] = norm_pool.tile([DK, NT], f32, tag="bc", name=f"bc_{h}")
                                nc.gpsimd.partition_broadcast(bcs[h][:], recs[h][:])
                            for h in hs:
                                po = (h % 2) * DK
                                nc.vector.tensor_mul(
                                    ctxT_h[th][po:po + DK, pair * TH + t0: pair * TH + t0 + NT],
                                    cps[h][0:DK, :],
                                    bcs[h][:],
                                )

                # tail: last t-quarter's output projection
                for _, u in out_units(TQ - 1):
                    u()

    nc.compile()
    return nc


def _get_nc():
    if "nc" not in _CACHE:
        _CACHE["nc"] = _build()
    return _CACHE["nc"]


def kernel(x, Wq, bq, Wk, bk, Wv, bv, Wo, bo):
    import ml_dtypes
    from concourse.bass_utils import run_bass_kernel_spmd

    bft = ml_dtypes.bfloat16
    x = np.ascontiguousarray(np.asarray(x, dtype=np.float32))
    Wq, Wk, Wv, Wo = (np.asarray(w, dtype=np.float32) for w in (Wq, Wk, Wv, Wo))
    bq, bk, bv, bo = (np.asarray(b, dtype=np.float32) for b in (bq, bk, bv, bo))

    nc = _get_nc()

    in_maps = []
    for c in range(NCORES):
        b = c // HPC
        g = c % HPC
        cols = slice(g * HD, (g + 1) * HD)
        in_maps.append({
            "xT": np.ascontiguousarray(x[b].T.astype(bft)),
            "wq": np.ascontiguousarray(Wq[:, cols].astype(bft)),
            "wk": np.ascontiguousarray(Wk[:, cols].astype(bft)),
            "wv": np.ascontiguousarray(Wv[:, cols].astype(bft)),
            "wo": np.ascontiguousarray(Wo[cols, :]),
            "bq": np.ascontiguousarray(bq[cols].reshape(MB, P).T),
            "bk": np.ascontiguousarray(bk[cols].reshape(MB, P).T),
        })

    res = run_bass_kernel_spmd(nc, in_maps, core_ids=list(range(NCORES)))
    _CACHE["last_result"] = res

    out = np.zeros((B, T, D), dtype=np.float32)
    for c in range(NCORES):
        out[c // HPC] += np.asarray(res.results[c]["out"], dtype=np.float32)
    # bv commutes with the softmax-normalized weighted sum: fold into bo.
    out += (bo + bv @ Wo)[None, None, :]
    return out



# revision 2
# speedup vs baseline: 1.0229x; 1.0229x over previous
"""Multi-head self-attention on 8 Trainium2 NeuronCores — v4 (ctx-flip).

Sharding: core c handles batch b = c // 4 and head-quad g = c % 4
(heads 4g..4g+3 = 256 of the 1024 projection columns). Each core runs
its heads' Q/K/V projections, attention, and a partial output projection
(contraction over its 256 context dims); the host sums 4 partials per
batch and adds bo' = bo + bv @ Wo (bv commutes with the normalized
softmax weights, so it folds into the output bias).

v4 key change vs v3: the context matmul is "flipped" — exp weights are
the stationary operand [128s x 128t] and V the moving operand
[128s, 64+1] bf16, costing 65 cycles instead of 512 per matmul (PE cost
scales with the moving free size only). The 8 per-(head, t-block)
accumulators share one PSUM bank via lazy zero-region semantics: the
first matmul of a round claims the bank (start=True) and every other
sub-tile's first write auto-zeroes its own bytes (start=False with
skip_group_check). Denominators accumulate via ones-column matmuls in a
second shared bank whose spare bytes also host the PE-transpose
outputs that restore ctx to [d, t] layout for the output projection.

Pipeline (per core):
  - ScalarE streams 128 exp instrs of [128, 1024] (~1.0us each) — the
    spine. PE per iteration: 2 score mms (427ns) + 16 flipped ctx mms
    (217ns); Q/K/V/out projections fill the remaining slack.
  - PSUM: scores 2x[128,1024] (4 banks) + ctx accum 1 + denom/transpose
    1 + filler 2 = 8 banks.
  - exp/V/ctxT/Wo all bf16 (backend forbids mixing 16/32-bit matmul
    operands; bf16 keeps full moving-rate and halves SBUF/DMA).
"""

import sys

sys.path.insert(0, "/opt/trn_rl_repo")

import numpy as np

B, T, D = 2, 2048, 1024
H = 16
DK = 64
NCORES = 8
HPC = 4            # heads per core
HD = HPC * DK      # 256 projection cols per core
P = 128
NT = 512           # matmul moving free dim
KB = D // P        # 8 contraction blocks for projections
MB = HD // P       # 2 col-blocks (head pairs) per core
SB = T // P        # 16 s-blocks
TQ = T // NT       # 4 t-quarters
VW = 65            # V columns per head incl. the denominator ones column
TH = T // 2

_CACHE = {}


def _build():
    import concourse.tile as tile
    from concourse import bacc, mybir

    f32 = mybir.dt.float32
    f32r = mybir.dt.float32r
    bf16 = mybir.dt.bfloat16
    Exp = mybir.ActivationFunctionType.Exp

    nc = bacc.Bacc("TRN2", target_bir_lowering=False, debug=False)

    xT_d = nc.dram_tensor("xT", [D, T], bf16, kind="ExternalInput").ap()
    wq_d = nc.dram_tensor("wq", [D, HD], bf16, kind="ExternalInput").ap()
    wk_d = nc.dram_tensor("wk", [D, HD], bf16, kind="ExternalInput").ap()
    wv_d = nc.dram_tensor("wv", [D, HD], bf16, kind="ExternalInput").ap()
    wo_d = nc.dram_tensor("wo", [HD, D], bf16, kind="ExternalInput").ap()
    bq_d = nc.dram_tensor("bq", [P, MB], f32, kind="ExternalInput").ap()
    bk_d = nc.dram_tensor("bk", [P, MB], f32, kind="ExternalInput").ap()
    id_d = nc.dram_tensor("ident", [P, P], bf16, kind="ExternalInput").ap()
    out_d = nc.dram_tensor("out", [T, D], bf16, kind="ExternalOutput").ap()

    with tile.TileContext(nc) as tc:
        with tc.tile_pool(name="persist", bufs=1) as persist:
            QT = persist.tile([P, MB * T], f32r, tag="qt")
            KT = persist.tile([P, MB * T], f32r, tag="kt")
            Vn = persist.tile([P, HPC * SB * VW], bf16, tag="vn")
            ctxT_h = [
                persist.tile([P, MB * TH], bf16, tag=f"ctxt{i}", name=f"ctxT_{i}")
                for i in range(2)
            ]
            wo_sb = persist.tile([P, MB * D], bf16, tag="wo")
            xT_sb = persist.tile([P, KB * T], bf16, tag="xt")
            id_sb = persist.tile([P, P], bf16, tag="id")
            w_sb = {
                name: persist.tile([P, KB * HD], bf16, tag=f"w{name}", name=f"w_{name}")
                for name in ("q", "k", "v")
            }
            b_sb = {
                name: persist.tile([P, MB], f32, tag=f"b{name}", name=f"b_{name}")
                for name in ("q", "k")
            }
            nc.gpsimd.memset(Vn[:], 1.0)
            # warm the Exp activation table at t~0 (ScalarE idles until the
            # first scores land; the auto-inserted table load would otherwise
            # cost 1.3us right before the first real exp)
            warm = persist.tile([1, 2], f32, tag="warm")
            nc.vector.memset(warm[:], 0.0)
            nc.scalar.activation(warm[:], warm[:], Exp)

            # ---- DMA issue order = consumption order ----
            nc.sync.dma_start(w_sb["q"][:, 0:HD], wq_d[0:P, :])
            nc.sync.dma_start(xT_sb[:, 0:T // 2], xT_d[0:P, 0:T // 2])
            nc.sync.dma_start(w_sb["k"][:, 0:HD], wk_d[0:P, :])
            nc.sync.dma_start(xT_sb[:, T // 2:T], xT_d[0:P, T // 2:T])
            nc.sync.dma_start(
                w_sb["q"][:, HD:].rearrange("p (k j) -> p k j", k=KB - 1),
                wq_d[P:, :].rearrange("(k p) j -> p k j", p=P),
            )
            nc.sync.dma_start(xT_sb[:, T:2 * T], xT_d[P:2 * P, :])
            nc.sync.dma_start(
                w_sb["k"][:, HD:].rearrange("p (k j) -> p k j", k=KB - 1),
                wk_d[P:, :].rearrange("(k p) j -> p k j", p=P),
            )
            for k in range(2, KB):
                nc.sync.dma_start(
                    xT_sb[:, k * T:(k + 1) * T], xT_d[k * P:(k + 1) * P, :]
                )
            nc.sync.dma_start(b_sb["q"][:], bq_d[:])
            nc.sync.dma_start(b_sb["k"][:], bk_d[:])
            nc.sync.dma_start(id_sb[:], id_d)
            nc.sync.dma_start(
                w_sb["v"][:].rearrange("p (k j) -> p k j", k=KB),
                wv_d.rearrange("(k p) j -> p k j", p=P),
            )
            for m in range(MB):
                nc.sync.dma_start(wo_sb[:, m * D:(m + 1) * D], wo_d[m * P:(m + 1) * P, :])

            dests = {"q": QT, "k": KT}

            def qk_evict(name, m, n, pst):
                dst = dests[name][:, m * T + n * NT: m * T + (n + 1) * NT]
                nc.vector.tensor_scalar_add(dst, pst[:], b_sb[name][:, m:m + 1])

            Vn_r = Vn[:].rearrange("p (h sw) -> p h sw", h=HPC)

            def v_units(pair, pool, tag, q4s, evict_eng="scalar"):
                # V in natural [s, d] layout: per psum tile 4 mt-blocks x 128
                # of the pair's d-cols; one unit per (tile, k) + one eviction
                # (a single 4D copy into Vn's 65-stride head blocks).
                for q4 in q4s:
                    ps = pool.tile([P, NT], f32, tag=tag, name=f"v_{pair}_{q4}")

                    # one accumulation group at a time per psum bank: each
                    # mt-block's k-sweep is a sequential group (427ns unit)
                    def mk(j, q4=q4, ps=ps):
                        def emit():
                            mt = q4 * 4 + j
                            for k in range(KB):
                                nc.tensor.matmul(
                                    ps[:, j * P:(j + 1) * P],
                                    xT_sb[:, k * T + mt * P: k * T + (mt + 1) * P],
                                    w_sb["v"][:, k * HD + pair * P: k * HD + (pair + 1) * P],
                                    start=(k == 0),
                                    stop=(k == KB - 1),
                                )
                        return 427, emit

                    for j in range(4):
                        yield mk(j)

                    def evict(q4=q4, ps=ps):
                        h0 = 2 * pair
                        dst = (
                            Vn_r[:, h0:h0 + 2, q4 * 4 * VW:(q4 + 1) * 4 * VW]
                            .rearrange("p h (j w) -> p h j w", j=4)[:, :, :, 0:DK]
                        )
                        # prefetch tiles evict on the idle ScalarE; in-loop
                        # tiles evict on DVE (ScalarE is exp-saturated there)
                        src_ap = ps[:].rearrange("p (j h w) -> p h j w", j=4, h=2)
                        if evict_eng == "scalar":
                            nc.scalar.copy(dst, src_ap)
                        else:
                            nc.vector.tensor_copy(dst, src_ap)
                    yield 0, evict

            # ---------------- Phase 1: pair0 Q/K sweeps ----------------
            with tc.tile_pool(name="qk0_ps", bufs=8, space="PSUM") as qk0_ps:
                # alloc order = slot order: V0/V1 (allocs 9/10) land on the
                # slots of (k,0)/(q,0), whose evicts run first on DVE.
                final_order = [("k", 0), ("q", 0), ("k", 1), ("k", 2),
                               ("k", 3), ("q", 1), ("q", 2), ("q", 3)]
                pst0 = {
                    (name, n): qk0_ps.tile([P, NT], f32, tag="qk0", name=f"p0_{name}_{n}")
                    for name, n in final_order
                }
                for k in range(KB):
                    pairs = (
                        final_order if k == KB - 1
                        else [(nm, n) for nm in ("q", "k") for n in range(TQ)]
                    )
                    for name, n in pairs:
                        nc.tensor.matmul(
                            pst0[(name, n)][:],
                            w_sb[name][:, k * HD: k * HD + P],
                            xT_sb[:, k * T + n * NT: k * T + (n + 1) * NT],
                            start=(k == 0),
                            stop=(k == KB - 1),
                        )
                # evict order: (k,n0)/(q,n0) unblock scores(sb0); K blocks
                # n1-3 are needed within tq0's first 16 iters; QT n1-3 later.
                qk_evict("k", 0, 0, pst0[("k", 0)])
                qk_evict("q", 0, 0, pst0[("q", 0)])
                for n in range(1, TQ):
                    qk_evict("k", 0, n, pst0[("k", n)])
                for n in range(1, TQ):
                    qk_evict("q", 0, n, pst0[("q", n)])
                # V(p0) tiles 0-1 reuse freed qk0 slots; tiles 2-3 go through
                # the fill chain inside the attention loop.
                for _, u in v_units(0, qk0_ps, "qk0", range(2)):
                    u()

            # ---------------- Phase 2: attention + filler ----------------
            with (
                tc.tile_pool(name="exp", bufs=8) as exp_pool,
                tc.tile_pool(name="norm", bufs=8) as norm_pool,
                tc.tile_pool(name="outsb", bufs=8) as out_sb_pool,
                tc.tile_pool(name="score_ps", bufs=2, space="PSUM") as score_ps,
                tc.tile_pool(name="ctx_ps", bufs=1, space="PSUM") as ctx_ps,
                tc.tile_pool(name="dn_ps", bufs=1, space="PSUM") as dn_ps,
                tc.tile_pool(name="fill_ps", bufs=2, space="PSUM") as fill_ps,
            ):
                # --- filler unit generators (each yields closures emitting
                # a small batch of PE work) ---
                def qk1_units(pairs):
                    # pair1 Q/K: k-pair units (2 matmuls, ~427ns) per (name, n)
                    for name, n in pairs:
                        if True:
                            ps = fill_ps.tile([P, NT], f32, tag="fill", name=f"qk1_{name}_{n}")

                            def mk(kp, name=name, n=n, ps=ps):
                                def emit():
                                    for k in (2 * kp, 2 * kp + 1):
                                        nc.tensor.matmul(
                                            ps[:],
                                            w_sb[name][:, k * HD + P: k * HD + 2 * P],
                                            xT_sb[:, k * T + n * NT: k * T + (n + 1) * NT],
                                            start=(k == 0),
                                            stop=(k == KB - 1),
                                        )
                                    if kp == KB // 2 - 1:
                                        qk_evict(name, 1, n, ps)
                                return 427, emit

                            for kp in range(KB // 2):
                                yield mk(kp)

                def out_units(tq, tail=False):
                    # output projection for t-quarter tq: one unit per
                    # (mt block, 512-col half) for smooth 427ns pacing.
                    for j in range(4):
                        mt = tq * 4 + j
                        th, tt = mt // 8, mt % 8
                        ot_box = []

                        ps_box = {}

                        def emit(ne, m, mt=mt, th=th, tt=tt, tq=tq,
                                 ot_box=ot_box, ps_box=ps_box):
                            if ne == 0 and m == 0:
                                ot_box.append(
                                    out_sb_pool.tile([P, D], bf16, tag="osb", name=f"osb_{mt}")
                                )
                            ot = ot_box[0]
                            if m == 0:
                                ps_box[ne] = fill_ps.tile(
                                    [P, NT], f32, tag="fill", name=f"o_{mt}_{ne}"
                                )
                            ps = ps_box[ne]
                            nc.tensor.matmul(
                                ps[:],
                                ctxT_h[th][:, m * TH + tt * P: m * TH + (tt + 1) * P],
                                wo_sb[:, m * D + ne * NT: m * D + (ne + 1) * NT],
                                start=(m == 0),
                                stop=(m == MB - 1),
                            )
                            if m == MB - 1:
                                if ne == 1 and tq >= TQ - 1:
                                    # after-loop tail only: ScalarE is idle
                                    # there. (GPSIMD cannot read PSUM.)
                                    nc.scalar.copy(ot[:, NT:], ps[:])
                                else:
                                    nc.vector.tensor_copy(ot[:, ne * NT:(ne + 1) * NT], ps[:])
                                if ne == 1:
                                    nc.sync.dma_start(out_d[mt * P:(mt + 1) * P, :], ot[:])
                        for ne in range(2):
                            for m in range(MB):
                                yield 213, (lambda ne=ne, m=m: emit(ne, m))

                import itertools

                # deadline order: V(p1) tile q is due at pair1 iter 4q+1,
                # KT(p1) block n at iter 64+4n+... ; QT(p1) n1-3 are due a
                # whole tq later, so they may spill into pair1-tq0's slack
                fill_p0 = itertools.chain(
                    v_units(0, fill_ps, "fill", range(2, 4), evict_eng="vector"),
                    qk1_units([("k", 0), ("q", 0)]),
                    v_units(1, fill_ps, "fill", [0]),
                    qk1_units([("k", 1)]),
                    v_units(1, fill_ps, "fill", [1]),
                    qk1_units([("k", 2)]),
                    v_units(1, fill_ps, "fill", [2]),
                    qk1_units([("k", 3)]),
                    v_units(1, fill_ps, "fill", [3]),
                    qk1_units([("q", 1), ("q", 2), ("q", 3)]),
                )
                p0_spent, P0_RATE = 0.0, 400.0

                for pair in range(MB):
                    hs = (2 * pair, 2 * pair + 1)
                    for tq in range(TQ):
                        th, t0 = tq // 2, (tq % 2) * NT
                        # shared-bank accumulators: 8 ctx sub-tiles [128, 64]
                        # in one bank; denominators + transpose scratch in a
                        # second bank.
                        cta = ctx_ps.tile([P, NT], f32, tag="cta", name="cta")
                        dnt = dn_ps.tile([P, NT], f32, tag="dnt", name="dnt")
                        fill_p1 = iter(out_units(tq - 1)) if (pair == 1 and tq > 0) else iter(())
                        p1_spent = 0.0

                        def ctx_emit(psb, pe):
                            # flipped ctx: stationary = exp [128s, 128t],
                            # moving = V [128s, 64] bf16 (+ ones col for the
                            # denominator into the dnt bank)
                            for i in range(2):
                                h = hs[i]
                                for tb in range(4):
                                    first = (psb == 0 and i == 0 and tb == 0)
                                    st = pe[:, i * NT + tb * P: i * NT + (tb + 1) * P]
                                    idx = i * 4 + tb
                                    nc.tensor.matmul(
                                        cta[:, idx * DK:(idx + 1) * DK],
                                        st,
                                        Vn[:, (h * SB + psb) * VW: (h * SB + psb) * VW + DK],
                                        start=first,
                                        stop=(psb == SB - 1),
                                        skip_group_check=not first,
                                    )
                                    nc.tensor.matmul(
                                        dnt[:, idx:idx + 1],
                                        st,
                                        Vn[:, (h * SB + psb) * VW + DK: (h * SB + psb + 1) * VW],
                                        start=first,
                                        stop=(psb == SB - 1),
                                        skip_group_check=not first,
                                    )

                        prev_exp = None
                        for sb in range(SB):
                            sc = score_ps.tile([P, 2 * NT], f32, tag="sc", name="sc")
                            for i, h in enumerate(hs):
                                po = (h % 2) * DK
                                nc.tensor.matmul(
                                    sc[:, i * NT:(i + 1) * NT],
                                    KT[po:po + DK, pair * T + sb * P: pair * T + (sb + 1) * P],
                                    QT[po:po + DK, pair * T + tq * NT:(pair * T) + (tq + 1) * NT],
                                    start=True,
                                    stop=True,
                                )
                            e = exp_pool.tile([P, 2 * NT], bf16, tag="exp", name="exp")
                            nc.scalar.activation(e[:], sc[:], Exp, scale=0.125)
                            # filler before ctx(prev): PE chews it while the
                            # previous iteration's exp completes
                            if pair == 0:
                                it = tq * SB + sb
                                while p0_spent < (it + 1) * P0_RATE:
                                    cu = next(fill_p0, None)
                                    if cu is None:
                                        break
                                    p0_spent += max(cu[0], 107)
                                    cu[1]()
                            else:
                                while p1_spent < (sb + 1) * 290:
                                    cu = next(fill_p0, None) or next(fill_p1, None)
                                    if cu is None:
                                        break
                                    p1_spent += max(cu[0], 107)
                                    cu[1]()
                            if prev_exp is not None:
                                ctx_emit(*prev_exp)
                            prev_exp = (sb, e)
                        ctx_emit(*prev_exp)
                        if pair == 1:
                            for _, u in fill_p1:
                                u()

                        # normalize + transpose this t-quarter into ctxT.
                        # Per t-block: both heads' recip+mul, two PE
                        # transposes into the dnt bank's spare bytes, one
                        # [128,128] bf16 eviction into ctxT.
                        for tb in range(4):
                            for i in range(2):
                                idx = i * 4 + tb
                                po = i * DK
                                rec = norm_pool.tile([P, 1], f32, tag="rec", name=f"rec_{idx}")
                                ctxn = norm_pool.tile([P, DK], bf16, tag="ctxn", name=f"ctxn_{idx}")
                                nc.vector.reciprocal(rec[:], dnt[:, idx:idx + 1])
                                nc.vector.tensor_scalar_mul(
                                    ctxn[:], cta[:, idx * DK:(idx + 1) * DK], rec[:]
                                )
                                tp = dnt[po:po + DK, P + tb * DK: P + (tb + 1) * DK].bitcast(bf16)
                                nc.tensor.matmul(
                                    tp, ctxn[:], id_sb[:], is_transpose=True,
                                    start=False, stop=True, skip_group_check=True,
                                    tile_position=(0, po),
                                )
                            nc.vector.tensor_copy(
                                ctxT_h[th][:, pair * TH + t0 + tb * P: pair * TH + t0 + (tb + 1) * P],
                                dnt[:, P + tb * DK: P + (tb + 1) * DK].bitcast(bf16),
                            )

                # tail: last t-quarter's output projection
                for _, u in out_units(TQ - 1, tail=True):
                    u()

    nc.compile()
    return nc


def _get_nc():
    if "nc" not in _CACHE:
        _CACHE["nc"] = _build()
    return _CACHE["nc"]


def kernel(x, Wq, bq, Wk, bk, Wv, bv, Wo, bo):
    import ml_dtypes
    from concourse.bass_utils import run_bass_kernel_spmd

    bft = ml_dtypes.bfloat16
    x = np.ascontiguousarray(np.asarray(x, dtype=np.float32))
    Wq, Wk, Wv, Wo = (np.asarray(w, dtype=np.float32) for w in (Wq, Wk, Wv, Wo))
    bq, bk, bv, bo = (np.asarray(b, dtype=np.float32) for b in (bq, bk, bv, bo))

    nc = _get_nc()

    ident = np.eye(P, dtype=bft)
    in_maps = []
    for c in range(NCORES):
        b = c // HPC
        g = c % HPC
        cols = slice(g * HD, (g + 1) * HD)
        in_maps.append({
            "xT": np.ascontiguousarray(x[b].T.astype(bft)),
            "wq": np.ascontiguousarray(Wq[:, cols].astype(bft)),
            "wk": np.ascontiguousarray(Wk[:, cols].astype(bft)),
            "wv": np.ascontiguousarray(Wv[:, cols].astype(bft)),
            "wo": np.ascontiguousarray(Wo[cols, :].astype(bft)),
            "bq": np.ascontiguousarray(bq[cols].reshape(MB, P).T),
            "bk": np.ascontiguousarray(bk[cols].reshape(MB, P).T),
            "ident": ident,
        })

    res = run_bass_kernel_spmd(nc, in_maps, core_ids=list(range(NCORES)))
    _CACHE["last_result"] = res

    out = np.zeros((B, T, D), dtype=np.float32)
    for c in range(NCORES):
        out[c // HPC] += np.asarray(res.results[c]["out"], dtype=np.float32)
    # bv commutes with the softmax-normalized weighted sum: fold into bo.
    out += (bo + bv @ Wo)[None, None, :]
    return out


# revision 8
# speedup vs baseline: 1.0560x; 1.0323x over previous
"""Multi-head self-attention on 8 Trainium2 NeuronCores — v4 (ctx-flip).

Sharding: core c handles batch b = c // 4 and head-quad g = c % 4
(heads 4g..4g+3 = 256 of the 1024 projection columns). Each core runs
its heads' Q/K/V projections, attention, and a partial output projection
(contraction over its 256 context dims); the host sums 4 partials per
batch and adds bo' = bo + bv @ Wo (bv commutes with the normalized
softmax weights, so it folds into the output bias).

v4 key change vs v3: the context matmul is "flipped" — exp weights are
the stationary operand [128s x 128t] and V the moving operand
[128s, 64+1] bf16, costing 65 cycles instead of 512 per matmul (PE cost
scales with the moving free size only). The 8 per-(head, t-block)
accumulators share one PSUM bank via lazy zero-region semantics: the
first matmul of a round claims the bank (start=True) and every other
sub-tile's first write auto-zeroes its own bytes (start=False with
skip_group_check). Denominators accumulate via ones-column matmuls in a
second shared bank whose spare bytes also host the PE-transpose
outputs that restore ctx to [d, t] layout for the output projection.

Pipeline (per core):
  - ScalarE streams 128 exp instrs of [128, 1024] (~1.0us each) — the
    spine. PE per iteration: 2 score mms (427ns) + 16 flipped ctx mms
    (217ns); Q/K/V/out projections fill the remaining slack.
  - PSUM: scores 2x[128,1024] (4 banks) + ctx accum 1 + denom/transpose
    1 + filler 2 = 8 banks.
  - exp/V/ctxT/Wo all bf16 (backend forbids mixing 16/32-bit matmul
    operands; bf16 keeps full moving-rate and halves SBUF/DMA).
"""

import sys

sys.path.insert(0, "/opt/trn_rl_repo")

import numpy as np

B, T, D = 2, 2048, 1024
H = 16
DK = 64
NCORES = 8
HPC = 4            # heads per core
HD = HPC * DK      # 256 projection cols per core
P = 128
NT = 512           # matmul moving free dim
KB = D // P        # 8 contraction blocks for projections
MB = HD // P       # 2 col-blocks (head pairs) per core
SB = T // P        # 16 s-blocks
TQ = T // NT       # 4 t-quarters
VW = 65            # V columns per head incl. the denominator ones column
TH = T // 2

_CACHE = {}


def _build():
    import concourse.tile as tile
    from concourse import bacc, mybir

    f32 = mybir.dt.float32
    f32r = mybir.dt.float32r
    bf16 = mybir.dt.bfloat16
    Exp = mybir.ActivationFunctionType.Exp

    nc = bacc.Bacc("TRN2", target_bir_lowering=False, debug=False)

    xT_d = nc.dram_tensor("xT", [D, T], bf16, kind="ExternalInput").ap()
    wq_d = nc.dram_tensor("wq", [D, HD], bf16, kind="ExternalInput").ap()
    wk_d = nc.dram_tensor("wk", [D, HD], bf16, kind="ExternalInput").ap()
    wv_d = nc.dram_tensor("wv", [D, HD], bf16, kind="ExternalInput").ap()
    wo_d = nc.dram_tensor("wo", [HD, D], bf16, kind="ExternalInput").ap()
    bq_d = nc.dram_tensor("bq", [P, MB], f32, kind="ExternalInput").ap()
    bk_d = nc.dram_tensor("bk", [P, MB], f32, kind="ExternalInput").ap()
    id_d = nc.dram_tensor("ident", [P, P], bf16, kind="ExternalInput").ap()
    out_d = nc.dram_tensor("out", [T, D], bf16, kind="ExternalOutput").ap()

    with tile.TileContext(nc) as tc:
        with tc.tile_pool(name="persist", bufs=1) as persist:
            QT = persist.tile([P, MB * T], f32r, tag="qt")
            KT = persist.tile([P, MB * T], f32r, tag="kt")
            Vn = persist.tile([P, HPC * SB * VW], bf16, tag="vn")
            ctxT_h = [
                persist.tile([P, MB * TH], bf16, tag=f"ctxt{i}", name=f"ctxT_{i}")
                for i in range(2)
            ]
            wo_sb = persist.tile([P, MB * D], bf16, tag="wo")
            xT_sb = persist.tile([P, KB * T], bf16, tag="xt")
            id_sb = persist.tile([P, P], bf16, tag="id")
            w_sb = {
                name: persist.tile([P, KB * HD], bf16, tag=f"w{name}", name=f"w_{name}")
                for name in ("q", "k", "v")
            }
            b_sb = {
                name: persist.tile([P, MB], f32, tag=f"b{name}", name=f"b_{name}")
                for name in ("q", "k")
            }
            nc.gpsimd.memset(Vn[:], 1.0)
            # warm the Exp activation table at t~0 (ScalarE idles until the
            # first scores land; the auto-inserted table load would otherwise
            # cost 1.3us right before the first real exp)
            warm = persist.tile([1, 2], f32, tag="warm")
            nc.vector.memset(warm[:], 0.0)
            nc.scalar.activation(warm[:], warm[:], Exp)

            # ---- DMA issue order = consumption order ----
            nc.sync.dma_start(w_sb["q"][:, 0:HD], wq_d[0:P, :])
            nc.sync.dma_start(xT_sb[:, 0:T // 2], xT_d[0:P, 0:T // 2])
            nc.sync.dma_start(w_sb["k"][:, 0:HD], wk_d[0:P, :])
            nc.sync.dma_start(xT_sb[:, T // 2:T], xT_d[0:P, T // 2:T])
            nc.sync.dma_start(
                w_sb["q"][:, HD:].rearrange("p (k j) -> p k j", k=KB - 1),
                wq_d[P:, :].rearrange("(k p) j -> p k j", p=P),
            )
            nc.sync.dma_start(xT_sb[:, T:2 * T], xT_d[P:2 * P, :])
            nc.sync.dma_start(
                w_sb["k"][:, HD:].rearrange("p (k j) -> p k j", k=KB - 1),
                wk_d[P:, :].rearrange("(k p) j -> p k j", p=P),
            )
            for k in range(2, KB):
                nc.sync.dma_start(
                    xT_sb[:, k * T:(k + 1) * T], xT_d[k * P:(k + 1) * P, :]
                )
            nc.sync.dma_start(b_sb["q"][:], bq_d[:])
            nc.sync.dma_start(b_sb["k"][:], bk_d[:])
            nc.sync.dma_start(id_sb[:], id_d)
            nc.sync.dma_start(
                w_sb["v"][:].rearrange("p (k j) -> p k j", k=KB),
                wv_d.rearrange("(k p) j -> p k j", p=P),
            )
            for m in range(MB):
                nc.sync.dma_start(wo_sb[:, m * D:(m + 1) * D], wo_d[m * P:(m + 1) * P, :])

            dests = {"q": QT, "k": KT}

            def qk_evict(name, m, n, pst):
                dst = dests[name][:, m * T + n * NT: m * T + (n + 1) * NT]
                nc.vector.tensor_scalar_add(dst, pst[:], b_sb[name][:, m:m + 1])

            Vn_r = Vn[:].rearrange("p (h sw) -> p h sw", h=HPC)

            def v_units(pair, pool, tag, q4s, evict_eng="scalar"):
                # V in natural [s, d] layout: per psum tile 4 mt-blocks x 128
                # of the pair's d-cols; one unit per (tile, k) + one eviction
                # (a single 4D copy into Vn's 65-stride head blocks).
                for q4 in q4s:
                    ps = pool.tile([P, NT], f32, tag=tag, name=f"v_{pair}_{q4}")

                    # one accumulation group at a time per psum bank: each
                    # mt-block's k-sweep is a sequential group (427ns unit)
                    def mk(j, q4=q4, ps=ps):
                        def emit():
                            mt = q4 * 4 + j
                            for k in range(KB):
                                nc.tensor.matmul(
                                    ps[:, j * P:(j + 1) * P],
                                    xT_sb[:, k * T + mt * P: k * T + (mt + 1) * P],
                                    w_sb["v"][:, k * HD + pair * P: k * HD + (pair + 1) * P],
                                    start=(k == 0),
                                    stop=(k == KB - 1),
                                )
                        return 427, emit

                    for j in range(4):
                        yield mk(j)

                    def evict(q4=q4, ps=ps):
                        h0 = 2 * pair
                        dst = (
                            Vn_r[:, h0:h0 + 2, q4 * 4 * VW:(q4 + 1) * 4 * VW]
                            .rearrange("p h (j w) -> p h j w", j=4)[:, :, :, 0:DK]
                        )
                        # prefetch tiles evict on the idle ScalarE; in-loop
                        # tiles evict on DVE (ScalarE is exp-saturated there)
                        src_ap = ps[:].rearrange("p (j h w) -> p h j w", j=4, h=2)
                        if evict_eng == "scalar":
                            nc.scalar.copy(dst, src_ap)
                        else:
                            nc.vector.tensor_copy(dst, src_ap)
                    yield 0, evict

            # ---------------- Phase 1: pair0 Q/K sweeps ----------------
            with tc.tile_pool(name="qk0_ps", bufs=8, space="PSUM") as qk0_ps:
                # alloc order = slot order: V0/V1 (allocs 9/10) land on the
                # slots of (k,0)/(q,0), whose evicts run first on DVE.
                final_order = [("k", 0), ("q", 0), ("k", 1), ("k", 2),
                               ("k", 3), ("q", 1), ("q", 2), ("q", 3)]
                pst0 = {
                    (name, n): qk0_ps.tile([P, NT], f32, tag="qk0", name=f"p0_{name}_{n}")
                    for name, n in final_order
                }
                for k in range(KB):
                    pairs = (
                        final_order if k == KB - 1
                        else [(nm, n) for nm in ("q", "k") for n in range(TQ)]
                    )
                    for name, n in pairs:
                        nc.tensor.matmul(
                            pst0[(name, n)][:],
                            w_sb[name][:, k * HD: k * HD + P],
                            xT_sb[:, k * T + n * NT: k * T + (n + 1) * NT],
                            start=(k == 0),
                            stop=(k == KB - 1),
                        )
                # evict order: (k,n0)/(q,n0) unblock scores(sb0); K blocks
                # n1-3 are needed within tq0's first 16 iters; QT n1-3 later.
                qk_evict("k", 0, 0, pst0[("k", 0)])
                qk_evict("q", 0, 0, pst0[("q", 0)])
                for n in range(1, TQ):
                    qk_evict("k", 0, n, pst0[("k", n)])
                for n in range(1, TQ):
                    qk_evict("q", 0, n, pst0[("q", n)])
                # V(p0) tiles 0-1 reuse freed qk0 slots; tiles 2-3 go through
                # the fill chain inside the attention loop.
                for _, u in v_units(0, qk0_ps, "qk0", range(2), evict_eng="vector"):
                    u()

            # ---------------- Phase 2: attention + filler ----------------
            with (
                tc.tile_pool(name="exp", bufs=8) as exp_pool,
                tc.tile_pool(name="norm", bufs=8) as norm_pool,
                tc.tile_pool(name="outsb", bufs=8) as out_sb_pool,
                tc.tile_pool(name="score_ps", bufs=2, space="PSUM") as score_ps,
                tc.tile_pool(name="ctx_ps", bufs=1, space="PSUM") as ctx_ps,
                tc.tile_pool(name="dn_ps", bufs=1, space="PSUM") as dn_ps,
                tc.tile_pool(name="fill_ps", bufs=2, space="PSUM") as fill_ps,
            ):
                # --- filler unit generators (each yields closures emitting
                # a small batch of PE work) ---
                def qk1_units(pairs):
                    # pair1 Q/K: k-pair units (2 matmuls, ~427ns) per (name, n)
                    for name, n in pairs:
                        if True:
                            ps = fill_ps.tile([P, NT], f32, tag="fill", name=f"qk1_{name}_{n}")

                            def mk(kp, name=name, n=n, ps=ps):
                                def emit():
                                    for k in (2 * kp, 2 * kp + 1):
                                        nc.tensor.matmul(
                                            ps[:],
                                            w_sb[name][:, k * HD + P: k * HD + 2 * P],
                                            xT_sb[:, k * T + n * NT: k * T + (n + 1) * NT],
                                            start=(k == 0),
                                            stop=(k == KB - 1),
                                        )
                                    if kp == KB // 2 - 1:
                                        qk_evict(name, 1, n, ps)
                                return 427, emit

                            for kp in range(KB // 2):
                                yield mk(kp)

                def out_units(tq, tail=False):
                    # output projection for t-quarter tq: one unit per
                    # (mt block, 512-col half) for smooth 427ns pacing.
                    for j in range(4):
                        mt = tq * 4 + j
                        th, tt = mt // 8, mt % 8
                        ot_box = []

                        ps_box = {}

                        def emit(ne, m, mt=mt, th=th, tt=tt, tq=tq,
                                 ot_box=ot_box, ps_box=ps_box):
                            if ne == 0 and m == 0:
                                ot_box.append(
                                    out_sb_pool.tile([P, D], bf16, tag="osb", name=f"osb_{mt}")
                                )
                            ot = ot_box[0]
                            if m == 0:
                                ps_box[ne] = fill_ps.tile(
                                    [P, NT], f32, tag="fill", name=f"o_{mt}_{ne}"
                                )
                            ps = ps_box[ne]
                            nc.tensor.matmul(
                                ps[:],
                                ctxT_h[th][:, m * TH + tt * P: m * TH + (tt + 1) * P],
                                wo_sb[:, m * D + ne * NT: m * D + (ne + 1) * NT],
                                start=(m == 0),
                                stop=(m == MB - 1),
                            )
                            if m == MB - 1:
                                if tail:
                                    # after-loop tail only: ScalarE is idle
                                    # there. (GPSIMD cannot read PSUM.)
                                    nc.scalar.copy(ot[:, ne * NT:(ne + 1) * NT], ps[:])
                                else:
                                    nc.vector.tensor_copy(ot[:, ne * NT:(ne + 1) * NT], ps[:])
                                # per-half DMA: the first half ships while the
                                # second is still projecting (shorter tail)
                                nc.sync.dma_start(
                                    out_d[mt * P:(mt + 1) * P, ne * NT:(ne + 1) * NT],
                                    ot[:, ne * NT:(ne + 1) * NT],
                                )
                        for ne in range(2):
                            for m in range(MB):
                                yield 213, (lambda ne=ne, m=m: emit(ne, m))

                import itertools

                # deadline order: V(p1) tile q is due at pair1 iter 4q+1,
                # KT(p1) block n at iter 64+4n+... ; QT(p1) n1-3 are due a
                # whole tq later, so they may spill into pair1-tq0's slack
                fill_p0 = itertools.chain(
                    v_units(0, fill_ps, "fill", range(2, 4), evict_eng="vector"),
                    qk1_units([("k", 0), ("q", 0)]),
                    v_units(1, fill_ps, "fill", [0]),
                    qk1_units([("k", 1)]),
                    v_units(1, fill_ps, "fill", [1]),
                    qk1_units([("k", 2)]),
                    v_units(1, fill_ps, "fill", [2]),
                    qk1_units([("k", 3)]),
                    v_units(1, fill_ps, "fill", [3]),
                    qk1_units([("q", 1), ("q", 2), ("q", 3)]),
                )
                p0_spent, P0_RATE = 0.0, 400.0

                def norm_units(pair, tq, th, t0, cta, dnt):
                    # normalize + transpose a finished t-quarter into ctxT.
                    # Yielded as units so they run as next-window filler
                    # instead of parking the in-order PE at the boundary.
                    for tb in range(4):
                        def dve_part(tb=tb):
                            for i in range(2):
                                idx = i * 4 + tb
                                rec = norm_pool.tile([P, 1], f32, tag="rec", name=f"rec_{idx}")
                                ctxn = norm_pool.tile([P, DK], bf16, tag="ctxn",
                                                      name=f"ctxn_{pair}_{tq}_{idx}")
                                nc.vector.reciprocal(rec[:], dnt[:, idx:idx + 1])
                                nc.vector.tensor_scalar_mul(
                                    ctxn[:], cta[:, idx * DK:(idx + 1) * DK], rec[:]
                                )
                                _CTXN[(pair, tq, idx)] = ctxn
                        yield 0, dve_part

                        def pe_part(tb=tb):
                            for i in range(2):
                                idx = i * 4 + tb
                                po = i * DK
                                ctxn = _CTXN.pop((pair, tq, idx))
                                tp = dnt[po:po + DK, P + tb * DK: P + (tb + 1) * DK].bitcast(bf16)
                                nc.tensor.matmul(
                                    tp, ctxn[:], id_sb[:], is_transpose=True,
                                    start=False, stop=True, skip_group_check=True,
                                    tile_position=(0, po),
                                )
                            nc.vector.tensor_copy(
                                ctxT_h[th][:, pair * TH + t0 + tb * P: pair * TH + t0 + (tb + 1) * P],
                                dnt[:, P + tb * DK: P + (tb + 1) * DK].bitcast(bf16),
                            )
                        yield 107, pe_part

                _CTXN = {}
                norm_carry = iter(())

                for pair in range(MB):
                    hs = (2 * pair, 2 * pair + 1)
                    for tq in range(TQ):
                        th, t0 = tq // 2, (tq % 2) * NT
                        # shared-bank accumulators: 8 ctx sub-tiles [128, 64]
                        # in one bank; denominators + transpose scratch in a
                        # second bank.
                        cta = ctx_ps.tile([P, NT], f32, tag="cta", name="cta")
                        dnt = dn_ps.tile([P, NT], f32, tag="dnt", name="dnt")
                        fill_p1 = iter(out_units(tq - 1)) if (pair == 1 and tq > 0) else iter(())
                        p1_spent = 0.0

                        def ctx_emit(psb, pe):
                            # flipped ctx: stationary = exp [128s, 128t],
                            # moving = V [128s, 64] bf16 (+ ones col for the
                            # denominator into the dnt bank)
                            for i in range(2):
                                h = hs[i]
                                for tb in range(4):
                                    first = (psb == 0 and i == 0 and tb == 0)
                                    st = pe[:, i * NT + tb * P: i * NT + (tb + 1) * P]
                                    idx = i * 4 + tb
                                    nc.tensor.matmul(
                                        cta[:, idx * DK:(idx + 1) * DK],
                                        st,
                                        Vn[:, (h * SB + psb) * VW: (h * SB + psb) * VW + DK],
                                        start=first,
                                        stop=(psb == SB - 1),
                                        skip_group_check=not first,
                                    )
                                    nc.tensor.matmul(
                                        dnt[:, idx:idx + 1],
                                        st,
                                        Vn[:, (h * SB + psb) * VW + DK: (h * SB + psb + 1) * VW],
                                        start=first,
                                        stop=(psb == SB - 1),
                                        skip_group_check=not first,
                                    )

                        # ctx matmuls lag LAG iterations behind their exp so
                        # the in-order PE never parks on a fresh exp's sem
                        # (the +~300ns ACT pipeline/sem latency would chain
                        # into the next scores and stretch the spine).
                        LAG = 3
                        pending = []
                        for sb in range(SB):
                            sc = score_ps.tile([P, 2 * NT], f32, tag="sc", name="sc")
                            for i, h in enumerate(hs):
                                po = (h % 2) * DK
                                nc.tensor.matmul(
                                    sc[:, i * NT:(i + 1) * NT],
                                    KT[po:po + DK, pair * T + sb * P: pair * T + (sb + 1) * P],
                                    QT[po:po + DK, pair * T + tq * NT:(pair * T) + (tq + 1) * NT],
                                    start=True,
                                    stop=True,
                                )
                            e = exp_pool.tile([P, 2 * NT], bf16, tag="exp", name="exp")
                            nc.scalar.activation(e[:], sc[:], Exp, scale=0.125)
                            # previous window's norm units first (their banks
                            # recycle at ctx(0), iter LAG), then filler
                            if sb < 2:
                                for _ in range(4):
                                    cu = next(norm_carry, None)
                                    if cu is None:
                                        break
                                    cu[1]()
                            # filler before ctx: PE chews it while exps flow
                            if pair == 0:
                                it = tq * SB + sb
                                while p0_spent < (it + 1) * P0_RATE:
                                    cu = next(fill_p0, None)
                                    if cu is None:
                                        break
                                    p0_spent += max(cu[0], 107)
                                    cu[1]()
                            else:
                                while p1_spent < (sb + 1) * 350:
                                    cu = next(fill_p0, None) or next(fill_p1, None)
                                    if cu is None:
                                        break
                                    p1_spent += max(cu[0], 107)
                                    cu[1]()
                            pending.append((sb, e))
                            if len(pending) > LAG:
                                ctx_emit(*pending.pop(0))
                        for pe_item in pending:
                            ctx_emit(*pe_item)
                        if pair == 1:
                            for _, u in fill_p1:
                                u()
                        # leftover norm units from the previous window must
                        # finish before this window's banks recycle
                        for _, u in norm_carry:
                            u()
                        norm_carry = iter(norm_units(pair, tq, th, t0, cta, dnt))

                # tail: last window's norm + output projection, interleaved
                # per t-block so the final out-proj starts ASAP
                tail_out = out_units(TQ - 1, tail=True)
                for tb in range(4):
                    for _ in range(2):
                        cost, u = next(norm_carry)
                        u()
                    for _ in range(4):
                        cu = next(tail_out, None)
                        if cu is not None:
                            cu[1]()
                for _, u in tail_out:
                    u()

    nc.compile()
    return nc


def _get_nc():
    if "nc" not in _CACHE:
        _CACHE["nc"] = _build()
    return _CACHE["nc"]


def kernel(x, Wq, bq, Wk, bk, Wv, bv, Wo, bo):
    import ml_dtypes
    from concourse.bass_utils import run_bass_kernel_spmd

    bft = ml_dtypes.bfloat16
    x = np.ascontiguousarray(np.asarray(x, dtype=np.float32))
    Wq, Wk, Wv, Wo = (np.asarray(w, dtype=np.float32) for w in (Wq, Wk, Wv, Wo))
    bq, bk, bv, bo = (np.asarray(b, dtype=np.float32) for b in (bq, bk, bv, bo))

    nc = _get_nc()

    ident = np.eye(P, dtype=bft)
    in_maps = []
    for c in range(NCORES):
        b = c // HPC
        g = c % HPC
        cols = slice(g * HD, (g + 1) * HD)
        in_maps.append({
            "xT": np.ascontiguousarray(x[b].T.astype(bft)),
            "wq": np.ascontiguousarray(Wq[:, cols].astype(bft)),
            "wk": np.ascontiguousarray(Wk[:, cols].astype(bft)),
            "wv": np.ascontiguousarray(Wv[:, cols].astype(bft)),
            "wo": np.ascontiguousarray(Wo[cols, :].astype(bft)),
            "bq": np.ascontiguousarray(bq[cols].reshape(MB, P).T),
            "bk": np.ascontiguousarray(bk[cols].reshape(MB, P).T),
            "ident": ident,
        })

    res = run_bass_kernel_spmd(nc, in_maps, core_ids=list(range(NCORES)))
    _CACHE["last_result"] = res

    out = np.zeros((B, T, D), dtype=np.float32)
    for c in range(NCORES):
        out[c // HPC] += np.asarray(res.results[c]["out"], dtype=np.float32)
    # bv commutes with the softmax-normalized weighted sum: fold into bo.
    out += (bo + bv @ Wo)[None, None, :]
    return out


# revision 15
# speedup vs baseline: 1.0993x; 1.0410x over previous
"""Multi-head self-attention on 8 Trainium2 NeuronCores — v5 (flat pipeline).

Sharding: core c handles batch b = c // 4 and head-quad g = c % 4
(heads 4g..4g+3 = 256 of the 1024 projection columns). Each core runs
its heads' Q/K/V projections, attention, and a partial output projection
(contraction over its 256 context dims); the host sums 4 partials per
batch and adds bo' = bo + bv @ Wo (bv commutes with the normalized
softmax weights, so it folds into the output bias).

Design (per core):
  - The spine is 128 iterations of (2 score mms -> one [128,1024] exp on
    ScalarE); ScalarE is ~saturated (~1.0us/iter).
  - Flipped ctx matmuls: exp is the stationary operand [128s x 128t] and
    V the moving [128s, 64(+1)] bf16 -> 65 cycles/mm instead of 512. The
    8 per-(head, t-block) accumulators share ONE psum bank via lazy
    zero-region semantics (first mm of a round start=True claims the
    bank; all others start=False + skip_group_check). Denominators via
    ones-column mms into a second bank whose spare bytes host the
    PE-transpose outputs that restore ctx to [d, t] for the out-proj.
  - ctx emission lags its exp by LAG iterations so the in-order PE never
    parks on a fresh exp's semaphore; the pipeline is flat across
    (pair, tq) windows with norm/transpose of window w running as filler
    inside window w+1.
  - All projection work (Q/K/V sweeps, out-proj) is deadline-scheduled
    filler inside the spine; only K/Q(pair0, tq0) run up front, paced by
    t-chunked xT DMAs so the spine starts ~9us in.
  - PSUM: scores 2x[128,1024] (4 banks) + ctx 1 + denom/transpose 1 +
    filler 2 = 8 banks. exp/V/ctxT/Wo are bf16 (the backend forbids
    mixed 16/32-bit matmul operands).
"""

import sys

sys.path.insert(0, "/opt/trn_rl_repo")

import numpy as np

B, T, D = 2, 2048, 1024
H = 16
DK = 64
NCORES = 8
HPC = 4            # heads per core
HD = HPC * DK      # 256 projection cols per core
P = 128
NT = 512           # matmul moving free dim
KB = D // P        # 8 contraction blocks for projections
MB = HD // P       # 2 col-blocks (head pairs) per core
SB = T // P        # 16 s-blocks
TQ = T // NT       # 4 t-quarters
VW = 65            # V columns per head incl. the denominator ones column
TH = T // 2
LAG = 3            # ctx emission lag (iterations) behind its exp

_CACHE = {}


def _build():
    import concourse.tile as tile
    from concourse import bacc, mybir

    f32 = mybir.dt.float32
    f32r = mybir.dt.float32r
    bf16 = mybir.dt.bfloat16
    Exp = mybir.ActivationFunctionType.Exp

    nc = bacc.Bacc("TRN2", target_bir_lowering=False, debug=False)

    xT_d = nc.dram_tensor("xT", [D, T], bf16, kind="ExternalInput").ap()
    wq_d = nc.dram_tensor("wq", [D, HD], bf16, kind="ExternalInput").ap()
    wk_d = nc.dram_tensor("wk", [D, HD], bf16, kind="ExternalInput").ap()
    wv_d = nc.dram_tensor("wv", [D, HD], bf16, kind="ExternalInput").ap()
    wo_d = nc.dram_tensor("wo", [HD, D], bf16, kind="ExternalInput").ap()
    bq_d = nc.dram_tensor("bq", [P, MB], f32, kind="ExternalInput").ap()
    bk_d = nc.dram_tensor("bk", [P, MB], f32, kind="ExternalInput").ap()
    id_d = nc.dram_tensor("ident", [P, P], bf16, kind="ExternalInput").ap()
    out_d = nc.dram_tensor("out", [T, D], bf16, kind="ExternalOutput").ap()
    import os
    DBG = bool(int(os.environ.get("KDBG", "0")))
    if DBG:
        dbg_qt = nc.dram_tensor("dbg_qt", [P, MB * T], f32, kind="ExternalOutput").ap()
        dbg_kt = nc.dram_tensor("dbg_kt", [P, MB * T], f32, kind="ExternalOutput").ap()
        dbg_vn = nc.dram_tensor("dbg_vn", [P, HPC * SB * VW], bf16, kind="ExternalOutput").ap()
        dbg_ct = [nc.dram_tensor(f"dbg_ct{i}", [P, MB * TH], bf16, kind="ExternalOutput").ap()
                  for i in range(2)]

    with tile.TileContext(nc) as tc:
        with tc.tile_pool(name="persist", bufs=1) as persist:
            QT = persist.tile([P, MB * T], f32r, tag="qt")
            KT = persist.tile([P, MB * T], f32r, tag="kt")
            Vn = persist.tile([P, HPC * SB * VW], bf16, tag="vn")
            ctxT_h = [
                persist.tile([P, MB * TH], bf16, tag=f"ctxt{i}", name=f"ctxT_{i}")
                for i in range(2)
            ]
            wo_sb = persist.tile([P, MB * D], bf16, tag="wo")
            xT_sb = persist.tile([P, KB * T], bf16, tag="xt")
            id_sb = persist.tile([P, P], bf16, tag="id")
            w_sb = {
                name: persist.tile([P, KB * HD], bf16, tag=f"w{name}", name=f"w_{name}")
                for name in ("q", "k", "v")
            }
            b_sb = {
                name: persist.tile([P, MB], f32, tag=f"b{name}", name=f"b_{name}")
                for name in ("q", "k")
            }
            nc.gpsimd.memset(Vn[:], 1.0)
            # warm the Exp activation table at t~0 (ScalarE idles until the
            # first scores land; the auto-inserted table load would otherwise
            # cost 1.3us right before the first real exp)
            warm = persist.tile([1, 2], f32, tag="warm")
            nc.vector.memset(warm[:], 0.0)
            nc.scalar.activation(warm[:], warm[:], Exp)

            # ---- DMA issue order = consumption order ----
            # xT streams in t-chunks (all k-blocks of a 512-t window per DMA)
            # so the first K/Q sweeps only wait for chunk 0; weights are
            # interleaved at their first-use points.
            xT_r = xT_sb[:].rearrange("p (k t) -> p k t", k=KB)
            xTd_r = xT_d.rearrange("(k p) t -> p k t", p=P)

            def xchunk(c, k0, k1):
                nc.sync.dma_start(
                    xT_r[:, k0:k1, c * NT:(c + 1) * NT],
                    xTd_r[:, k0:k1, c * NT:(c + 1) * NT],
                )

            def wdma(name, dram):
                nc.sync.dma_start(w_sb[name][:, 0:HD], dram[0:P, :])
                nc.sync.dma_start(
                    w_sb[name][:, HD:].rearrange("p (k j) -> p k j", k=KB - 1),
                    dram[P:, :].rearrange("(k p) j -> p k j", p=P),
                )

            wdma("k", wk_d)
            xchunk(0, 0, 4)
            xchunk(0, 4, 8)
            wdma("q", wq_d)
            nc.sync.dma_start(b_sb["k"][:], bk_d[:])
            nc.sync.dma_start(b_sb["q"][:], bq_d[:])
            wdma("v", wv_d)
            xchunk(1, 0, 8)
            xchunk(2, 0, 8)
            xchunk(3, 0, 8)
            nc.sync.dma_start(id_sb[:], id_d)
            for m in range(MB):
                nc.sync.dma_start(wo_sb[:, m * D:(m + 1) * D], wo_d[m * P:(m + 1) * P, :])

            dests = {"q": QT, "k": KT}

            def qk_evict(name, m, n, pst):
                dst = dests[name][:, m * T + n * NT: m * T + (n + 1) * NT]
                nc.vector.tensor_scalar_add(dst, pst[:], b_sb[name][:, m:m + 1])

            Vn_r = Vn[:].rearrange("p (h sw) -> p h sw", h=HPC)

            # ---------------- pools ----------------
            with (
                tc.tile_pool(name="exp", bufs=8) as exp_pool,
                tc.tile_pool(name="norm", bufs=8) as norm_pool,
                tc.tile_pool(name="outsb", bufs=8) as out_sb_pool,
                tc.tile_pool(name="score_ps", bufs=2, space="PSUM") as score_ps,
                tc.tile_pool(name="ctx_ps", bufs=1, space="PSUM") as ctx_ps,
                tc.tile_pool(name="dn_ps", bufs=1, space="PSUM") as dn_ps,
                tc.tile_pool(name="fill_ps", bufs=2, space="PSUM") as fill_ps,
            ):
                # --- filler unit generators ---
                def qk_units(name, pair, n):
                    # one Q/K projection sweep: 4 units of 2 matmuls (427ns),
                    # eviction rides the last unit
                    ps = fill_ps.tile([P, NT], f32, tag="fill", name=f"qk_{name}_{pair}_{n}")

                    def mk(kp):
                        def emit():
                            for k in (2 * kp, 2 * kp + 1):
                                nc.tensor.matmul(
                                    ps[:],
                                    w_sb[name][:, k * HD + pair * P: k * HD + (pair + 1) * P],
                                    xT_sb[:, k * T + n * NT: k * T + (n + 1) * NT],
                                    start=(k == 0),
                                    stop=(k == KB - 1),
                                )
                            if kp == KB // 2 - 1:
                                qk_evict(name, pair, n, ps)
                        return emit

                    for kp in range(KB // 2):
                        yield 427, mk(kp)

                def v_units(pair, q4, evict_eng="vector"):
                    # V projection for s-blocks 4*q4..4*q4+3 of a pair:
                    # 4 units (427ns) + eviction
                    ps = fill_ps.tile([P, NT], f32, tag="fill", name=f"v_{pair}_{q4}")

                    def mk(j):
                        def emit():
                            mt = q4 * 4 + j
                            for k in range(KB):
                                nc.tensor.matmul(
                                    ps[:, j * P:(j + 1) * P],
                                    xT_sb[:, k * T + mt * P: k * T + (mt + 1) * P],
                                    w_sb["v"][:, k * HD + pair * P: k * HD + (pair + 1) * P],
                                    start=(k == 0),
                                    stop=(k == KB - 1),
                                )
                        return emit

                    for j in range(4):
                        yield 427, mk(j)

                    def evict():
                        h0 = 2 * pair
                        dst = (
                            Vn_r[:, h0:h0 + 2, q4 * 4 * VW:(q4 + 1) * 4 * VW]
                            .rearrange("p h (j w) -> p h j w", j=4)[:, :, :, 0:DK]
                        )
                        src_ap = ps[:].rearrange("p (j h w) -> p h j w", j=4, h=2)
                        if evict_eng == "scalar":
                            nc.scalar.copy(dst, src_ap)
                        else:
                            nc.vector.tensor_copy(dst, src_ap)
                    yield 0, evict

                def out_units(w, tail=False):
                    # output projection for window w=(pair, tq): one unit per
                    # (mt block, 512-col half, m) for smooth 213ns pacing
                    pair, tq = divmod(w, TQ)
                    if pair == 0:
                        return
                    for j in range(4):
                        mt = tq * 4 + j
                        th, tt = mt // 8, mt % 8
                        ot_box = []
                        ps_box = {}

                        def emit(ne, m, mt=mt, th=th, tt=tt,
                                 ot_box=ot_box, ps_box=ps_box):
                            if ne == 0 and m == 0:
                                ot_box.append(
                                    out_sb_pool.tile([P, D], bf16, tag="osb", name=f"osb_{mt}")
                                )
                            ot = ot_box[0]
                            if m == 0:
                                ps_box[ne] = fill_ps.tile(
                                    [P, NT], f32, tag="fill", name=f"o_{mt}_{ne}"
                                )
                            ps = ps_box[ne]
                            nc.tensor.matmul(
                                ps[:],
                                ctxT_h[th][:, m * TH + tt * P: m * TH + (tt + 1) * P],
                                wo_sb[:, m * D + ne * NT: m * D + (ne + 1) * NT],
                                start=(m == 0),
                                stop=(m == MB - 1),
                            )
                            if m == MB - 1:
                                if tail:
                                    nc.scalar.copy(ot[:, ne * NT:(ne + 1) * NT], ps[:])
                                else:
                                    nc.vector.tensor_copy(ot[:, ne * NT:(ne + 1) * NT], ps[:])
                                # per-half DMA shortens the tail
                                nc.sync.dma_start(
                                    out_d[mt * P:(mt + 1) * P, ne * NT:(ne + 1) * NT],
                                    ot[:, ne * NT:(ne + 1) * NT],
                                )
                        for ne in range(2):
                            for m in range(MB):
                                yield 213, (lambda ne=ne, m=m: emit(ne, m))

                _CTXN = {}

                def norm_units(w, cta, dnt):
                    # normalize + transpose a finished window into ctxT;
                    # runs as next-window filler so the in-order PE never
                    # parks at the boundary.
                    pair, tq = divmod(w, TQ)
                    th, t0 = tq // 2, (tq % 2) * NT
                    for tb in range(4):
                        def dve_part(tb=tb):
                            for i in range(2):
                                idx = i * 4 + tb
                                rec = norm_pool.tile([P, 1], f32, tag="rec", name=f"rec_{idx}")
                                ctxn = norm_pool.tile([P, DK], bf16, tag="ctxn",
                                                      name=f"ctxn_{w}_{idx}")
                                nc.vector.reciprocal(rec[:], dnt[:, idx:idx + 1])
                                nc.vector.tensor_scalar_mul(
                                    ctxn[:], cta[:, idx * DK:(idx + 1) * DK], rec[:]
                                )
                                _CTXN[(w, idx)] = ctxn
                        yield 0, dve_part

                        def pe_part(tb=tb):
                            for i in range(2):
                                idx = i * 4 + tb
                                po = i * DK
                                ctxn = _CTXN.pop((w, idx))
                                tp = dnt[po:po + DK, P + tb * DK: P + (tb + 1) * DK].bitcast(bf16)
                                nc.tensor.matmul(
                                    tp, ctxn[:], id_sb[:], is_transpose=True,
                                    start=False, stop=True, skip_group_check=True,
                                    tile_position=(0, po),
                                )
                            nc.vector.tensor_copy(
                                ctxT_h[th][:, pair * TH + t0 + tb * P: pair * TH + t0 + (tb + 1) * P],
                                dnt[:, P + tb * DK: P + (tb + 1) * DK].bitcast(bf16),
                            )
                        yield 107, pe_part

                # --- deadline-driven filler schedule ---
                # (deadline_iter, seq, iterator) entries; per iteration we
                # drain everything due soon, then optional units by deadline.
                import heapq

                sched = []
                seq_counter = [0]

                def add_sched(deadline, gen):
                    heapq.heappush(sched, (deadline, seq_counter[0], iter(gen)))
                    seq_counter[0] += 1

                spent = [0.0]

                def drain(now_iter, budget_ns):
                    # forced: everything with deadline <= now+1; optional:
                    # greedy by deadline while under budget
                    while sched:
                        deadline, s, g = sched[0]
                        forced = deadline <= now_iter + 1
                        if not forced and spent[0] >= budget_ns:
                            break
                        item = next(g, None)
                        if item is None:
                            heapq.heappop(sched)
                            continue
                        cost, emit = item
                        emit()
                        spent[0] += max(cost, 80)

                # projection filler deadlines (global iteration index)
                for c in range(1, TQ):
                    add_sched(4 * c - 1, qk_units("k", 0, c))
                for c in range(TQ):
                    add_sched(4 * c + 2, v_units(0, c))
                for c in range(1, TQ):
                    add_sched(16 * c - 2, qk_units("q", 0, c))
                for c in range(TQ):
                    add_sched(64 + 4 * c - 3, qk_units("k", 1, c))
                    add_sched(64 + 4 * c + 1, v_units(1, c))
                add_sched(64 - 2, qk_units("q", 1, 0))
                for c in range(1, TQ):
                    add_sched(64 + 16 * c - 2, qk_units("q", 1, c))

                # ---------------- phase 1: K/Q(pair0, n0) ----------------
                for name in ("k", "q"):
                    for _, u in qk_units(name, 0, 0):
                        u()

                # ---------------- the 128-iteration spine ----------------
                pending = []      # (w, sb, exp_tile) awaiting ctx emission
                norm_carry = iter(())
                ctx_state = {}    # w -> (cta, dnt)

                def ctx_emit(w, psb, pe):
                    pair, tq = divmod(w, TQ)
                    hs = (2 * pair, 2 * pair + 1)
                    if psb == 0:
                        # previous window's norm must fully emit before its
                        # banks are reclaimed by this window's first mms
                        for _, u in norm_carry_box[0]:
                            u()
                        ctx_state[w] = (
                            ctx_ps.tile([P, NT], f32, tag="cta", name="cta"),
                            dn_ps.tile([P, NT], f32, tag="dnt", name="dnt"),
                        )
                    cta, dnt = ctx_state[w]
                    for i in range(2):
                        h = hs[i]
                        for tb in range(4):
                            first = (psb == 0 and i == 0 and tb == 0)
                            st = pe[:, i * NT + tb * P: i * NT + (tb + 1) * P]
                            idx = i * 4 + tb
                            nc.tensor.matmul(
                                cta[:, idx * DK:(idx + 1) * DK],
                                st,
                                Vn[:, (h * SB + psb) * VW: (h * SB + psb) * VW + DK],
                                start=first,
                                stop=(psb == SB - 1),
                                skip_group_check=not first,
                            )
                            nc.tensor.matmul(
                                dnt[:, idx:idx + 1],
                                st,
                                Vn[:, (h * SB + psb) * VW + DK: (h * SB + psb + 1) * VW],
                                start=first,
                                stop=(psb == SB - 1),
                                skip_group_check=not first,
                            )
                    if psb == SB - 1:
                        import itertools
                        cta, dnt = ctx_state.pop(w)
                        nu = iter(norm_units(w, cta, dnt))
                        norm_carry_box[0] = nu
                        if w != 7:  # window 7's out-proj runs in the tail
                            # chain norm ahead of out-proj so the scheduler
                            # can never emit an out-proj read before the
                            # ctxT writes it depends on exist (emission
                            # order = dependency order); the shared iterator
                            # means units still emit exactly once
                            add_sched(16 * w + 16 + LAG + 3,
                                      itertools.chain(nu, out_units(w)))

                norm_carry_box = [norm_carry]

                for it in range(8 * SB):
                    w, sb = divmod(it, SB)
                    pair, tq = divmod(w, TQ)
                    hs = (2 * pair, 2 * pair + 1)
                    sc = score_ps.tile([P, 2 * NT], f32, tag="sc", name="sc")
                    for i, h in enumerate(hs):
                        po = (h % 2) * DK
                        nc.tensor.matmul(
                            sc[:, i * NT:(i + 1) * NT],
                            KT[po:po + DK, pair * T + sb * P: pair * T + (sb + 1) * P],
                            QT[po:po + DK, pair * T + tq * NT:(pair * T) + (tq + 1) * NT],
                            start=True,
                            stop=True,
                        )
                        # ramp-friendly: don't let scores overtake exp wildly
                    e = exp_pool.tile([P, 2 * NT], bf16, tag="exp", name="exp")
                    nc.scalar.activation(e[:], sc[:], Exp, scale=0.125)
                    # a couple of lagged norm units (cheap, due quickly)
                    for _ in range(3):
                        cu = next(norm_carry_box[0], None)
                        if cu is None:
                            break
                        cu[1]()
                    # deadline filler
                    drain(it, (it + 1) * 400.0)
                    # lagged ctx
                    pending.append((w, sb, e))
                    if len(pending) > LAG:
                        ctx_emit(*pending.pop(0))

                # ---------------- tail ----------------
                # drain remaining ctx, then interleave the last window's
                # norm with its output projection per t-block
                for item in pending:
                    ctx_emit(*item)
                drain(10 ** 9, float("inf"))
                tail_out = out_units(7, tail=True)
                nt_iter = norm_carry_box[0]
                for tb in range(4):
                    for _ in range(2):
                        cu = next(nt_iter, None)
                        if cu is not None:
                            cu[1]()
                    for _ in range(4):
                        cu = next(tail_out, None)
                        if cu is not None:
                            cu[1]()
                for _, u in nt_iter:
                    u()
                for _, u in tail_out:
                    u()
                if DBG:
                    nc.sync.dma_start(dbg_qt, QT[:].bitcast(f32))
                    nc.sync.dma_start(dbg_kt, KT[:].bitcast(f32))
                    nc.sync.dma_start(dbg_vn, Vn[:])
                    for i in range(2):
                        nc.sync.dma_start(dbg_ct[i], ctxT_h[i][:])

    nc.compile()
    return nc


def _get_nc():
    if "nc" not in _CACHE:
        _CACHE["nc"] = _build()
    return _CACHE["nc"]


def kernel(x, Wq, bq, Wk, bk, Wv, bv, Wo, bo):
    import ml_dtypes
    from concourse.bass_utils import run_bass_kernel_spmd

    bft = ml_dtypes.bfloat16
    x = np.ascontiguousarray(np.asarray(x, dtype=np.float32))
    Wq, Wk, Wv, Wo = (np.asarray(w, dtype=np.float32) for w in (Wq, Wk, Wv, Wo))
    bq, bk, bv, bo = (np.asarray(b, dtype=np.float32) for b in (bq, bk, bv, bo))

    nc = _get_nc()

    ident = np.eye(P, dtype=bft)
    in_maps = []
    for c in range(NCORES):
        b = c // HPC
        g = c % HPC
        cols = slice(g * HD, (g + 1) * HD)
        in_maps.append({
            "xT": np.ascontiguousarray(x[b].T.astype(bft)),
            "wq": np.ascontiguousarray(Wq[:, cols].astype(bft)),
            "wk": np.ascontiguousarray(Wk[:, cols].astype(bft)),
            "wv": np.ascontiguousarray(Wv[:, cols].astype(bft)),
            "wo": np.ascontiguousarray(Wo[cols, :].astype(bft)),
            "bq": np.ascontiguousarray(bq[cols].reshape(MB, P).T),
            "bk": np.ascontiguousarray(bk[cols].reshape(MB, P).T),
            "ident": ident,
        })

    res = run_bass_kernel_spmd(nc, in_maps, core_ids=list(range(NCORES)))
    _CACHE["last_result"] = res

    out = np.zeros((B, T, D), dtype=np.float32)
    for c in range(NCORES):
        out[c // HPC] += np.asarray(res.results[c]["out"], dtype=np.float32)
    # bv commutes with the softmax-normalized weighted sum: fold into bo.
    out += (bo + bv @ Wo)[None, None, :]
    return out
